# revision 7
# baseline (speedup 1.0000x reference)
"""Trainium2 Bass kernel for nn_Attention (B=4, N=2048, C=768, H=12).

Returns (out, attention) like the reference:
  qkv = x @ qkv_w.T -> q,k,v ; attn = softmax(q k^T / sqrt(dh)) ; out = (attn v) @ proj_w.T + proj_b

Sharding over 8 cores: core c -> batch b = c//2, head-group hg = c%2 (6 heads).
Each core computes its 6 heads' attention matrix [6, 2048, 2048] plus a partial
projection output [2048, 768] (summed pairwise on host; host adds proj_b).

Per-core dataflow (matmuls in float32r; P@V in bf16):
  phase 1: load xT/weights, compute Q^T,K^T ([d, n] layout) and V ([m, d] layout,
           augmented with a ones-column for softmax sums).
  per head:
    S^T[m, n] = K^T lhsT @ Q^T rhs   (PE)  -> exp(0.125 s) -> E^T bf16 in SBUF (ACT)
    PV: out[n-tile, 65] = E^T-slice lhsT @ V_aug rhs (bf16, PSUM-accum over m-tiles)
        col 64 = softmax denominators (ones column trick)
    recip = 1/sums (DVE); ctx[n, d] = out * recip (DVE); Ln(recip) -> bias (ACT)
    S[n, m] = Q^T lhsT @ K^T rhs (PE) -> P = exp(0.125 s + ln recip) (ACT, fused
        normalize via per-partition bias) -> DMA to attention output
  tail: ctx tiles PE-transposed -> ctx^T; proj partial = ctx^T lhsT @ wpT rhs.

PSUM budget (16KB/partition = 8 banks): pool "big" [128,1024]x3 (6 banks) shared
by qkv/S^T/S chunks; pool "small" [128,384]x2 (2 banks) shared by V/PV/ctx-T/proj.
"""

import numpy as np

import concourse.bass as bass
import concourse.tile as tile
from concourse import bacc, mybir
from concourse.masks import make_identity

B, N, C = 4, 2048, 768
H, DH = 12, 64
G = 6                 # heads per core
NT = N // 128         # 16 n-tiles (also m-tiles)
F32 = mybir.dt.float32
F32R = mybir.dt.float32r
BF16 = mybir.dt.bfloat16
EXP = mybir.ActivationFunctionType.Exp
LN = mybir.ActivationFunctionType.Ln
SCALE = DH ** -0.5


def build_nc(repeats: int = 1):
    nc = bacc.Bacc(None, target_bir_lowering=False)

    xT_d = nc.dram_tensor("xT", [C, N], F32R, kind="ExternalInput")
    wqkT_d = nc.dram_tensor("wqkT", [C, 2 * G * DH], F32R, kind="ExternalInput")
    wvT_d = nc.dram_tensor("wvT", [C, G * DH], F32R, kind="ExternalInput")
    wpT_d = nc.dram_tensor("wpT", [G * DH, C], F32R, kind="ExternalInput")
    attn_d = nc.dram_tensor("attn", [G, N, N], F32, kind="ExternalOutput")
    outp_d = nc.dram_tensor("outp", [N, C], F32, kind="ExternalOutput")

    KC = C // 128       # 6 contraction tiles over channels
    QK = 2 * G * DH     # 768 q|k output channels
    CT = G * DH // 128  # 3 ctx channel tiles

    with tile.TileContext(nc) as tc:
        with (
            tc.tile_pool(name="persist", bufs=1) as persist,
            tc.tile_pool(name="qk", bufs=1) as qkpool,
            tc.tile_pool(name="psb", bufs=3, space=bass.MemorySpace.PSUM) as psb,
            tc.tile_pool(name="pss", bufs=2, space=bass.MemorySpace.PSUM) as pss,
        ):
            def big_ps():
                return psb.tile([128, 1024], F32, tag="big", name="bigps")

            def small_ps(shape, dt=F32):
                return pss.tile(shape, dt, tag="small", name="smallps")

            ident = persist.tile([128, 128], F32)
            make_identity(nc, ident)

            qkT = qkpool.tile([128, KC, N], F32R, tag="qkT")      # Q^T rows 0-2, K^T rows 3-5
            vaug = persist.tile([128, NT, G, DH + 1], BF16)
            wpT = persist.tile([128, CT, C], F32R)
            recip = persist.tile([128, G, NT], F32)
            neglog = persist.tile([128, G, NT], F32)
            ctxT = persist.tile([128, CT, N], F32R)               # [c, n]

            for _ in range(repeats):
                with tc.tile_pool(name="loads", bufs=1) as loads:
                    xT = loads.tile([128, KC, N], F32R)
                    wqkT = loads.tile([128, KC, QK], F32R)
                    wvT = loads.tile([128, KC, G * DH], F32R)
                    nc.sync.dma_start(out=xT[:], in_=xT_d[:].rearrange("(t p) n -> p t n", p=128))
                    nc.sync.dma_start(out=wqkT[:], in_=wqkT_d[:].rearrange("(t p) o -> p t o", p=128))
                    nc.sync.dma_start(out=wvT[:], in_=wvT_d[:].rearrange("(t p) o -> p t o", p=128))
                    nc.sync.dma_start(out=wpT[:], in_=wpT_d[:].rearrange("(t p) o -> p t o", p=128))

                    # ---- phase 1a: qkv^T = wqkT.T @ xT  -> Q^T|K^T [768, 2048]
                    for mo in range(QK // 128):
                        for nh in range(2):
                            ps = big_ps()
                            for kc in range(KC):
                                for ch in range(2):
                                    nc.tensor.matmul(
                                        ps[:, ch * 512:(ch + 1) * 512],
                                        wqkT[:, kc, mo * 128:(mo + 1) * 128],
                                        xT[:, kc, nh * 1024 + ch * 512: nh * 1024 + (ch + 1) * 512],
                                        start=(kc == 0), stop=(kc == KC - 1),
                                    )
                            nc.vector.tensor_copy(qkT[:, mo, nh * 1024:(nh + 1) * 1024], ps[:])

                    # ---- phase 1b: V[m, d] = xT.T @ wvT  (+ ones column)
                    for mt in range(NT):
                        ps = small_ps([128, G * DH])
                        for kc in range(KC):
                            nc.tensor.matmul(
                                ps[:],
                                xT[:, kc, mt * 128:(mt + 1) * 128],
                                wvT[:, kc, :],
                                start=(kc == 0), stop=(kc == KC - 1),
                            )
                        nc.vector.tensor_copy(
                            vaug[:, mt, :, 0:DH],
                            ps[:].rearrange("p (g d) -> p g d", g=G),
                        )
                    nc.vector.memset(vaug[:, :, :, DH:DH + 1], 1.0)

                # ---- phase 2: per head
                phase2 = tc.tile_pool(name="phase2", bufs=1)
                etpool = phase2.__enter__()
                ppool_cm = tc.tile_pool(name="pout", bufs=2)
                ppool = ppool_cm.__enter__()
                stage_cm = tc.tile_pool(name="stage", bufs=3)
                stage = stage_cm.__enter__()

                def qh(h):   # Q^T head slice [64, N]
                    return qkT[(h % 2) * 64:(h % 2) * 64 + 64, h // 2, :]

                def kh(h):   # K^T head slice [64, N]
                    return qkT[(h % 2) * 64:(h % 2) * 64 + 64, 3 + h // 2, :]

                for h in range(G):
                    et = etpool.tile([128, NT, N], BF16, tag="et")
                    # S^T[m-tile, n] + exp -> E^T (bf16)
                    for mt in range(NT):
                        for nh in range(2):
                            ps = big_ps()
                            for ch in range(2):
                                nc.tensor.matmul(
                                    ps[:, ch * 512:(ch + 1) * 512],
                                    kh(h)[:, mt * 128:(mt + 1) * 128],
                                    qh(h)[:, nh * 1024 + ch * 512: nh * 1024 + (ch + 1) * 512],
                                    start=True, stop=True,
                                )
                            nc.scalar.activation(
                                et[:, mt, nh * 1024:(nh + 1) * 1024], ps[:], EXP, scale=SCALE
                            )

                    # PV: out[n-tile, dh+1] accumulated over m-tiles (bf16)
                    for nt in range(NT):
                        ps = small_ps([128, DH + 1])
                        for mt in range(NT):
                            nc.tensor.matmul(
                                ps[:],
                                et[:, mt, nt * 128:(nt + 1) * 128],
                                vaug[:, mt, h, :],
                                start=(mt == 0), stop=(mt == NT - 1),
                            )
                        nc.vector.reciprocal(recip[:, h, nt:nt + 1], ps[:, DH:DH + 1])
                        cstage = stage.tile([128, DH], F32, tag="ctx")
                        nc.vector.tensor_scalar_mul(cstage[:], ps[:, 0:DH], recip[:, h, nt:nt + 1])
                        # transpose ctx tile -> ctxT[c, n]
                        pst = small_ps([DH, 128])
                        nc.tensor.transpose(pst[:], cstage[:], ident[:])
                        nc.vector.tensor_copy(
                            ctxT[(h % 2) * 64:(h % 2) * 64 + 64, h // 2, nt * 128:(nt + 1) * 128],
                            pst[:],
                        )

                    nc.scalar.activation(neglog[:, h, :], recip[:, h, :], LN)

                    # S[n-tile, m] -> P = exp(scale*s + ln(recip)) -> DMA out
                    for nt in range(NT):
                        p_sb = ppool.tile([128, N], F32, tag="p")
                        for half in range(2):
                            ps = big_ps()
                            for ch in range(2):
                                nc.tensor.matmul(
                                    ps[:, ch * 512:(ch + 1) * 512],
                                    qh(h)[:, nt * 128:(nt + 1) * 128],
                                    kh(h)[:, half * 1024 + ch * 512: half * 1024 + (ch + 1) * 512],
                                    start=True, stop=True,
                                )
                            nc.scalar.activation(
                                p_sb[:, half * 1024:(half + 1) * 1024], ps[:], EXP,
                                bias=neglog[:, h, nt:nt + 1], scale=SCALE,
                            )
                        nc.sync.dma_start(
                            out=attn_d[h, nt * 128:(nt + 1) * 128, :], in_=p_sb[:]
                        )

                # ---- phase 3: proj partial = ctxT.T @ wpT
                for nt in range(NT):
                    o_sb = stage.tile([128, C], F32, tag="osb")
                    for half in range(2):
                        ps = small_ps([128, C // 2])
                        for ct in range(CT):
                            nc.tensor.matmul(
                                ps[:],
                                ctxT[:, ct, nt * 128:(nt + 1) * 128],
                                wpT[:, ct, half * 384:(half + 1) * 384],
                                start=(ct == 0), stop=(ct == CT - 1),
                            )
                        nc.vector.tensor_copy(o_sb[:, half * 384:(half + 1) * 384], ps[:])
                    nc.sync.dma_start(out=outp_d[nt * 128:(nt + 1) * 128, :], in_=o_sb[:])
                stage_cm.__exit__(None, None, None)
                ppool_cm.__exit__(None, None, None)
                phase2.__exit__(None, None, None)

    nc.compile()
    return nc


_NC_CACHE = {}


def _get_nc(repeats: int = 1):
    if repeats not in _NC_CACHE:
        _NC_CACHE[repeats] = build_nc(repeats)
    return _NC_CACHE[repeats]


def make_in_maps(x, qkv_w, proj_w):
    x = np.ascontiguousarray(np.asarray(x, dtype=np.float32))
    qkv_w = np.asarray(qkv_w, dtype=np.float32)
    proj_w = np.asarray(proj_w, dtype=np.float32)
    wq, wk, wv = qkv_w[0:C], qkv_w[C:2 * C], qkv_w[2 * C:3 * C]
    in_maps = []
    for c in range(8):
        b, hg = c // 2, c % 2
        rows = slice(hg * G * DH, (hg + 1) * G * DH)
        in_maps.append({
            "xT": np.ascontiguousarray(x[b].T),
            "wqkT": np.ascontiguousarray(
                np.concatenate([wq[rows].T, wk[rows].T], axis=1)),
            "wvT": np.ascontiguousarray(wv[rows].T),
            "wpT": np.ascontiguousarray(proj_w[:, rows].T),
        })
    return in_maps


def run_on_cores(nc, in_maps, **kwargs):
    from concourse.bass_utils import run_bass_kernel_spmd
    return run_bass_kernel_spmd(nc, in_maps, list(range(8)), **kwargs)


def assemble(results, proj_b):
    attention = np.empty((B, H, N, N), dtype=np.float32)
    out = np.empty((B, N, C), dtype=np.float32)
    proj_b = np.asarray(proj_b, dtype=np.float32)
    for c in range(8):
        b, hg = c // 2, c % 2
        attention[b, hg * G:(hg + 1) * G] = results[c]["attn"]
    for b in range(B):
        out[b] = results[2 * b]["outp"] + results[2 * b + 1]["outp"] + proj_b
    return out, attention


def kernel(x, qkv_w, proj_w, proj_b):
    assert np.asarray(x).shape == (B, N, C)
    nc = _get_nc(1)
    in_maps = make_in_maps(x, qkv_w, proj_w)
    res = run_on_cores(nc, in_maps)
    return assemble(res.results, proj_b)


# revision 12
# speedup vs baseline: 457.2573x; 457.2573x over previous
"""Trainium2 Bass kernel for nn_Attention (B=4, N=2048, C=768, H=12).

Returns (out, attention) like the reference:
  qkv = x @ qkv_w.T -> q,k,v ; attn = softmax(q k^T / sqrt(dh)) ; out = (attn v) @ proj_w.T + proj_b

Sharding over 8 cores: core c -> batch b = c//2, head-group hg = c%2 (6 heads).
Each core computes its 6 heads' attention matrix [6, 2048, 2048] plus a partial
projection output [2048, 768] (summed pairwise on host; host adds proj_b).

Per-core dataflow (matmuls in float32r; P@V in bf16):
  phase 1: load xT/weights, compute Q^T,K^T ([d, n] layout) and V ([m, d] layout,
           augmented with a ones-column for softmax sums).
  per head:
    S^T[m, n] = K^T lhsT @ Q^T rhs   (PE)  -> exp(0.125 s) -> E^T bf16 in SBUF (ACT)
    PV: out[n-tile, 65] = E^T-slice lhsT @ V_aug rhs (bf16, PSUM-accum over m-tiles)
        col 64 = softmax denominators (ones column trick)
    recip = 1/sums (DVE); ctx[n, d] = out * recip (DVE); Ln(recip) -> bias (ACT)
    S[n, m] = Q^T lhsT @ K^T rhs (PE) -> P = exp(0.125 s + ln recip) (ACT, fused
        normalize via per-partition bias) -> DMA to attention output
  tail: ctx tiles PE-transposed -> ctx^T; proj partial = ctx^T lhsT @ wpT rhs.

PSUM budget (16KB/partition = 8 banks): pool "big" [128,1024]x3 (6 banks) shared
by qkv/S^T/S chunks; pool "small" [128,384]x2 (2 banks) shared by V/PV/ctx-T/proj.
"""

import numpy as np

import concourse.bass as bass
import concourse.tile as tile
from concourse import bacc, mybir
from concourse.masks import make_identity

B, N, C = 4, 2048, 768
H, DH = 12, 64
G = 6                 # heads per core
NT = N // 128         # 16 n-tiles (also m-tiles)
F32 = mybir.dt.float32
F32R = mybir.dt.float32r
BF16 = mybir.dt.bfloat16
EXP = mybir.ActivationFunctionType.Exp
LN = mybir.ActivationFunctionType.Ln
SCALE = DH ** -0.5
ATTN_DT = BF16          # DRAM attention dtype; host upcasts to fp32


def build_nc(repeats: int = 1, timing: bool = False):
    nc = bacc.Bacc(None, target_bir_lowering=False)

    if timing:
        # internal scratch I/O: no host transfers, wrapped in a HW loop
        xT_d = nc.dram_tensor("xT", [C, N], F32R)
        wqkT_d = nc.dram_tensor("wqkT", [C, 2 * G * DH], F32R)
        wvT_d = nc.dram_tensor("wvT", [C, G * DH], F32R)
        wpT_d = nc.dram_tensor("wpT", [G * DH, C], F32R)
        attn_d = nc.dram_tensor("attn", [G, N, N], ATTN_DT)
        outp_d = nc.dram_tensor("outp", [N, C], F32)
        tiny_d = nc.dram_tensor("tiny", [4], F32, kind="ExternalOutput")
    else:
        xT_d = nc.dram_tensor("xT", [C, N], F32R, kind="ExternalInput")
        wqkT_d = nc.dram_tensor("wqkT", [C, 2 * G * DH], F32R, kind="ExternalInput")
        wvT_d = nc.dram_tensor("wvT", [C, G * DH], F32R, kind="ExternalInput")
        wpT_d = nc.dram_tensor("wpT", [G * DH, C], F32R, kind="ExternalInput")
        attn_d = nc.dram_tensor("attn", [G, N, N], ATTN_DT, kind="ExternalOutput")
        outp_d = nc.dram_tensor("outp", [N, C], F32, kind="ExternalOutput")

    KC = C // 128       # 6 contraction tiles over channels
    QK = 2 * G * DH     # 768 q|k output channels
    CT = G * DH // 128  # 3 ctx channel tiles

    with tile.TileContext(nc) as tc:
        with (
            tc.tile_pool(name="persist", bufs=1) as persist,
            tc.tile_pool(name="qk", bufs=1) as qkpool,
            tc.tile_pool(name="psb", bufs=1, space=bass.MemorySpace.PSUM) as psb,
            tc.tile_pool(name="psst", bufs=1, space=bass.MemorySpace.PSUM) as psst,
            tc.tile_pool(name="pss", bufs=2, space=bass.MemorySpace.PSUM) as pss,
        ):
            def big_ps():
                return psb.tile([128, 1024], F32, tag="big", name="bigps")

            def st_ps():
                return psst.tile([128, 2048], F32, tag="st", name="stps")

            def small_ps(shape, dt=F32):
                return pss.tile(shape, dt, tag="small", name="smallps")

            ident = persist.tile([128, 128], F32)
            make_identity(nc, ident)

            qkT = qkpool.tile([128, KC, N], F32R, tag="qkT")      # Q^T rows 0-2, K^T rows 3-5
            vaug = persist.tile([128, NT, G, DH + 1], BF16)
            wpT = persist.tile([128, CT, C], F32R)
            recip = persist.tile([128, G, NT], F32)
            ctxT = persist.tile([128, CT, N], F32R)               # [c, n]

            from contextlib import ExitStack, nullcontext
            if timing:
                cnt = persist.tile([1, 4], F32, name="cnt")
                nc.vector.memset(cnt[:], 0.0)
            rep_cm = tc.For_i(0, repeats, 1) if timing else nullcontext()
            with rep_cm:
                if timing:
                    nc.vector.tensor_scalar_add(cnt[:], cnt[:], 1.0)
                with tc.tile_pool(name="loads", bufs=1) as loads:
                    xT = loads.tile([128, KC, N], F32R)
                    wqkT = loads.tile([128, KC, QK], F32R)
                    wvT = loads.tile([128, KC, G * DH], F32R)
                    nc.sync.dma_start(out=xT[:], in_=xT_d[:].rearrange("(t p) n -> p t n", p=128))
                    nc.sync.dma_start(out=wqkT[:], in_=wqkT_d[:].rearrange("(t p) o -> p t o", p=128))
                    nc.sync.dma_start(out=wvT[:], in_=wvT_d[:].rearrange("(t p) o -> p t o", p=128))
                    nc.sync.dma_start(out=wpT[:], in_=wpT_d[:].rearrange("(t p) o -> p t o", p=128))

                    # ---- phase 1a: qkv^T = wqkT.T @ xT  -> Q^T|K^T [768, 2048]
                    for mo in range(QK // 128):
                        for nh in range(2):
                            ps = big_ps()
                            for kc in range(KC):
                                for ch in range(2):
                                    nc.tensor.matmul(
                                        ps[:, ch * 512:(ch + 1) * 512],
                                        wqkT[:, kc, mo * 128:(mo + 1) * 128],
                                        xT[:, kc, nh * 1024 + ch * 512: nh * 1024 + (ch + 1) * 512],
                                        start=(kc == 0), stop=(kc == KC - 1),
                                    )
                            nc.vector.tensor_copy(qkT[:, mo, nh * 1024:(nh + 1) * 1024], ps[:])

                    # ---- phase 1b: V[m, d] = xT.T @ wvT  (+ ones column)
                    for mt in range(NT):
                        ps = small_ps([128, G * DH])
                        for kc in range(KC):
                            nc.tensor.matmul(
                                ps[:],
                                xT[:, kc, mt * 128:(mt + 1) * 128],
                                wvT[:, kc, :],
                                start=(kc == 0), stop=(kc == KC - 1),
                            )
                        nc.vector.tensor_copy(
                            vaug[:, mt, :, 0:DH],
                            ps[:].rearrange("p (g d) -> p g d", g=G),
                        )
                    nc.vector.memset(vaug[:, :, :, DH:DH + 1], 1.0)

                # ---- phase 2: per head
                phase2 = tc.tile_pool(name="phase2", bufs=1)
                etpool = phase2.__enter__()
                ppool_cm = tc.tile_pool(name="pout", bufs=2)
                ppool = ppool_cm.__enter__()
                stage_cm = tc.tile_pool(name="stage", bufs=3)
                stage = stage_cm.__enter__()

                def qh(h):   # Q^T head slice [64, N]
                    return qkT[(h % 2) * 64:(h % 2) * 64 + 64, h // 2, :]

                def kh(h):   # K^T head slice [64, N]
                    return qkT[(h % 2) * 64:(h % 2) * 64 + 64, 3 + h // 2, :]

                for h in range(G):
                    et = etpool.tile([128, NT, N], BF16, tag="et")
                    # S^T[m-tile, n] + exp -> E^T (bf16)
                    for mt in range(NT):
                        ps = st_ps()
                        for ch in range(4):
                            nc.tensor.matmul(
                                ps[:, ch * 512:(ch + 1) * 512],
                                kh(h)[:, mt * 128:(mt + 1) * 128],
                                qh(h)[:, ch * 512:(ch + 1) * 512],
                                start=True, stop=True,
                            )
                        nc.scalar.activation(et[:, mt, :], ps[:], EXP, scale=SCALE)

                    # PV: out[n-tile, dh+1] accumulated over m-tiles (bf16)
                    for nt in range(NT):
                        ps = small_ps([128, DH + 1])
                        for mt in range(NT):
                            nc.tensor.matmul(
                                ps[:],
                                et[:, mt, nt * 128:(nt + 1) * 128],
                                vaug[:, mt, h, :],
                                start=(mt == 0), stop=(mt == NT - 1),
                            )
                        nc.vector.reciprocal(recip[:, h, nt:nt + 1], ps[:, DH:DH + 1])
                        cstage = stage.tile([128, DH], F32, tag="ctx")
                        nc.vector.tensor_scalar_mul(cstage[:], ps[:, 0:DH], recip[:, h, nt:nt + 1])
                        # transpose ctx tile -> ctxT[c, n]
                        pst = small_ps([DH, 128])
                        nc.tensor.transpose(pst[:], cstage[:], ident[:])
                        nc.vector.tensor_copy(
                            ctxT[(h % 2) * 64:(h % 2) * 64 + 64, h // 2, nt * 128:(nt + 1) * 128],
                            pst[:],
                        )

                    # S[n-tile, m] -> E = exp(scale*s) -> P = E*recip (bf16) -> DMA
                    for nt in range(NT):
                        e_sb = ppool.tile([128, N], F32, tag="e")
                        for half in range(2):
                            ps = big_ps()
                            for ch in range(2):
                                nc.tensor.matmul(
                                    ps[:, ch * 512:(ch + 1) * 512],
                                    qh(h)[:, nt * 128:(nt + 1) * 128],
                                    kh(h)[:, half * 1024 + ch * 512: half * 1024 + (ch + 1) * 512],
                                    start=True, stop=True,
                                )
                            nc.scalar.activation(
                                e_sb[:, half * 1024:(half + 1) * 1024], ps[:], EXP,
                                scale=SCALE,
                            )
                        p_sb = ppool.tile([128, N], ATTN_DT, tag="p")
                        nc.vector.tensor_scalar_mul(p_sb[:], e_sb[:], recip[:, h, nt:nt + 1])
                        nc.sync.dma_start(
                            out=attn_d[h, nt * 128:(nt + 1) * 128, :], in_=p_sb[:]
                        )

                # ---- phase 3: proj partial = ctxT.T @ wpT
                for nt in range(NT):
                    o_sb = stage.tile([128, C], F32, tag="osb")
                    for half in range(2):
                        ps = small_ps([128, C // 2])
                        for ct in range(CT):
                            nc.tensor.matmul(
                                ps[:],
                                ctxT[:, ct, nt * 128:(nt + 1) * 128],
                                wpT[:, ct, half * 384:(half + 1) * 384],
                                start=(ct == 0), stop=(ct == CT - 1),
                            )
                        nc.vector.tensor_copy(o_sb[:, half * 384:(half + 1) * 384], ps[:])
                    nc.sync.dma_start(out=outp_d[nt * 128:(nt + 1) * 128, :], in_=o_sb[:])
                stage_cm.__exit__(None, None, None)
                ppool_cm.__exit__(None, None, None)
                phase2.__exit__(None, None, None)
            if timing:
                nc.sync.dma_start(out=tiny_d[:], in_=cnt[0])

    nc.compile()
    return nc


_NC_CACHE = {}


def _get_nc(repeats: int = 1):
    if repeats not in _NC_CACHE:
        _NC_CACHE[repeats] = build_nc(repeats)
    return _NC_CACHE[repeats]


def make_in_maps(x, qkv_w, proj_w):
    x = np.ascontiguousarray(np.asarray(x, dtype=np.float32))
    qkv_w = np.asarray(qkv_w, dtype=np.float32)
    proj_w = np.asarray(proj_w, dtype=np.float32)
    wq, wk, wv = qkv_w[0:C], qkv_w[C:2 * C], qkv_w[2 * C:3 * C]
    in_maps = []
    for c in range(8):
        b, hg = c // 2, c % 2
        rows = slice(hg * G * DH, (hg + 1) * G * DH)
        in_maps.append({
            "xT": np.ascontiguousarray(x[b].T),
            "wqkT": np.ascontiguousarray(
                np.concatenate([wq[rows].T, wk[rows].T], axis=1)),
            "wvT": np.ascontiguousarray(wv[rows].T),
            "wpT": np.ascontiguousarray(proj_w[:, rows].T),
        })
    return in_maps


def run_on_cores(nc, in_maps, **kwargs):
    from concourse.bass_utils import run_bass_kernel_spmd
    return run_bass_kernel_spmd(nc, in_maps, list(range(8)), **kwargs)


def assemble(results, proj_b):
    attention = np.empty((B, H, N, N), dtype=np.float32)
    out = np.empty((B, N, C), dtype=np.float32)
    proj_b = np.asarray(proj_b, dtype=np.float32)
    for c in range(8):
        b, hg = c // 2, c % 2
        attention[b, hg * G:(hg + 1) * G] = np.asarray(results[c]["attn"]).astype(np.float32)
    for b in range(B):
        out[b] = results[2 * b]["outp"] + results[2 * b + 1]["outp"] + proj_b
    return out, attention


def kernel(x, qkv_w, proj_w, proj_b):
    assert np.asarray(x).shape == (B, N, C)
    nc = _get_nc(1)
    in_maps = make_in_maps(x, qkv_w, proj_w)
    res = run_on_cores(nc, in_maps)
    return assemble(res.results, proj_b)
